# revision 25
# baseline (speedup 1.0000x reference)
"""Trainium2 Bass kernel for nn_NestRQModel (NEST-RQ pretraining loss).

Strategy: data-parallel over COMPACTED valid CE rows.  The reference only
counts rows (b, j) with j+1 < lengths[b]//4 (5700 of 8176); the host builds
that row list from feats_lengths (pure reindexing) and pads to 6144 = 8
cores x 768 rows.  Each core computes partial masked sums (nll, corr) and a
vocab-presence scatter; the host combines into the 4 scalar outputs.

Per-core pipeline (768 rows = 6 row tiles):
  stage 0: LayerNorm stats (bn_stats); projection matmul with LN folded in
           afterwards as a per-row affine fix (LN commutes through the
           linear map); rstd = exp(-0.5*ln(var+eps)) on ACT, with all Ln's
           batched before all Exp's (one table switch, no Sqrt set);
           x packed to bf16 hi/lo Kahan rows for stage A.
  stage A: dots = x @ E^T as K=128 Kahan bf16 matmuls (error ~1e-6, full
           PE rate); argmax runs DIRECTLY ON PSUM: per 1024-chunk
           max8/find_index8, then a vectorized cross-chunk combine.
  stage B: logits = enc @ W bf16 (full PE rate); ACT Exp with accum_out
           -> row sum-of-exp; corr via a SECOND sharp exp on ACT:
           sum_v exp(S*(logit_v - thr)) < 0.5  <=>  no logit above
           thr = ltgt + margin  (overflow -> inf -> counted incorrect,
           which is the right answer);  tgt logit = indirect-DMA gather
           of W^T rows + fused dot (scalar_tensor_tensor accum).
  PSUM is split: stage-A chunks and stage-B chunks each get their own
  2-buffer [128,1024] pool so DVE scans never block PE/ACT progress.
  presence: indirect-DMA scatter of 1.0 at (code*mask) indices.
"""
import os
import sys

import numpy as np

os.environ.setdefault("MYCRO_LOCAL_CACHE", "1")

try:
    import concourse.bass as bass
except ImportError:
    sys.path.insert(0, "/opt/trn_rl_repo")
    import concourse.bass as bass

import ml_dtypes
import concourse.bacc as bacc
import concourse.tile as tile
from concourse import mybir
from concourse.bass import IndirectOffsetOnAxis
from concourse.masks import make_identity
from contextlib import ExitStack

F32 = mybir.dt.float32
BF16 = mybir.dt.bfloat16
U32 = mybir.dt.uint32
I32 = mybir.dt.int32
AF = mybir.ActivationFunctionType
ALU = mybir.AluOpType

# problem constants
NCORES = 8
B, T, F = 16, 2048, 80
STK, STRIDE = 4, 4
N = 512                 # frames per batch after subsampling
SF = STK * F            # 320 stacked feature dim
EDIM = 16
V = 8192
D = 512                 # encoder dim
R = 768                 # compacted rows per core (8*768 = 6144 >= 5700)
RT = R // 128           # 6 row tiles
VC = 1024               # vocab/psum chunk (2 PSUM banks)
NVC = V // VC           # 8
EPS_LN = 1e-6
MARGIN = 7e-3           # corr margin: covers bf16 matmul noise (~1.5e-3)
SHARP = 400.0           # corr sharp-exp scale
K_LIST = [int(x) for x in os.environ.get(
    "NESTRQ_KLIST", "0,0,0,0,3,6").split(",")]  # corr chunks on DVE/tile

_NC_CACHE = {}


def _build_program():
    if "nc" in _NC_CACHE:
        return _NC_CACHE["nc"]
    nc = bacc.Bacc("TRN2", target_bir_lowering=False)

    # stk_all = [stacked feats | maskce] packed; pjp = [pj0|pj1|pj2pad|csum]
    stack_rows = nc.declare_dram_parameter("stack_rows", [128, RT * SF + RT],
                                           F32, isOutput=False)
    st01 = nc.declare_dram_parameter("st01", [128, 2 * R], F32, isOutput=False)
    st2 = nc.declare_dram_parameter("st2", [64, R], F32, isOutput=False)
    pjp = nc.declare_dram_parameter("pjp", [128, 4 * EDIM], F32, isOutput=False)
    ek = nc.declare_dram_parameter("Ek", [64, V], BF16, isOutput=False)
    w = nc.declare_dram_parameter("W", [D, V], BF16, isOutput=False)
    wt = nc.declare_dram_parameter("Wt", [V, D], F32, isOutput=False)
    encT = nc.declare_dram_parameter("encT", [128, 4 * R], BF16, isOutput=False)
    enc_rows = nc.declare_dram_parameter("enc_rows", [128, RT * D], BF16, isOutput=False)

    out_stats = nc.declare_dram_parameter("out_stats", [1, 8], F32, isOutput=True)
    out_pres = nc.declare_dram_parameter("out_pres", [V, 1], F32, isOutput=True)

    with tile.TileContext(nc) as tc, ExitStack() as ctx:
        const_p = ctx.enter_context(tc.tile_pool(name="const", bufs=1))
        small_p = ctx.enter_context(tc.tile_pool(name="small", bufs=4))
        cols_p = ctx.enter_context(tc.tile_pool(name="cols", bufs=1))
        stage0_p = ctx.enter_context(tc.tile_pool(name="stage0", bufs=2))
        scans_p = ctx.enter_context(tc.tile_pool(name="scans", bufs=3))
        scr_p = ctx.enter_context(tc.tile_pool(name="scr", bufs=2))
        psa_p = ctx.enter_context(tc.tile_pool(name="psa", bufs=2, space="PSUM"))
        psb_p = ctx.enter_context(tc.tile_pool(name="psb", bufs=2, space="PSUM"))

        # stage-0 path: small direct DMAs split across sync/scalar queues
        stk_all = const_p.tile([128, RT * SF + RT], F32)
        nc.sync.dma_start(stk_all[:], stack_rows[:])
        st01_sb = const_p.tile([128, 2 * R], F32)
        nc.scalar.dma_start(st01_sb[:], st01[:])
        st2_sb = const_p.tile([64, R], F32)
        nc.scalar.dma_start(st2_sb[:], st2[:])
        pjp_sb = const_p.tile([128, 4 * EDIM], F32)
        nc.sync.dma_start(pjp_sb[:], pjp[:])
        st_sb = [st01_sb[:, 0:R], st01_sb[0:128, R:2 * R], st2_sb[:]]
        pj = [pjp_sb[:, 0:EDIM], pjp_sb[:, EDIM:2 * EDIM],
              pjp_sb[0:64, 2 * EDIM:3 * EDIM]]
        csum_b = pjp_sb[:, 3 * EDIM:4 * EDIM]
        mask_all = stk_all[:, RT * SF:RT * SF + RT]

        ident = const_p.tile([128, 128], F32)
        make_identity(nc, ident[:])
        eps_t = const_p.tile([128, 1], F32)
        nc.vector.memset(eps_t[:], EPS_LN)
        ones_t = const_p.tile([128, 1], F32)
        nc.vector.memset(ones_t[:], 1.0)
        # iota8x8 = [0, 8, 16, ..., 56] per partition
        iota8 = const_p.tile([128, 8], F32)
        for h in range(8):
            nc.vector.memset(iota8[:, h:h + 1], float(8 * h))

        # heavy loads via SWDGE indirect gathers (fast descriptor gen)
        rowi = []
        for kc in range(4):
            it = const_p.tile([128, 1], U32, name=f"rowi_{kc}")
            nc.gpsimd.iota(it[:], pattern=[[0, 1]], base=128 * kc,
                           channel_multiplier=1)
            rowi.append(it)

        # Ek transferred dense [64,V] = [Eh|El|Eh|El]; expanded into the
        # 4 SBUF quadrants (junk rows pair with xk's zero rows)
        ek_sb = const_p.tile([128, V], BF16)
        iota32 = const_p.tile([32, 1], U32)
        nc.gpsimd.iota(iota32[:], pattern=[[0, 1]], base=0,
                       channel_multiplier=1)
        m16 = const_p.tile([32, 1], U32)
        nc.vector.tensor_scalar(m16[:], iota32[:], 16, None, ALU.is_lt)
        ekoff = []
        for q in range(4):
            ot = const_p.tile([32, 1], U32, name=f"ekoff_{q}")
            nc.vector.tensor_scalar(ot[:], iota32[:], 16 * q, None, ALU.add)
            nc.vector.tensor_tensor(out=ot[:], in0=ot[:], in1=m16[:],
                                    op=ALU.mult)
            ekoff.append(ot)
        for q in range(4):
            nc.gpsimd.indirect_dma_start(
                out=ek_sb[32 * q:32 * (q + 1), :], out_offset=None, in_=ek[:],
                in_offset=IndirectOffsetOnAxis(ap=ekoff[q][:, :1], axis=0))
        # Serialize: later gathers' offset tiles depend on ek_sb data, so
        # their transfers only start once Ek (stage-A critical) has landed.
        ekdep = const_p.tile([128, 1], U32)
        nc.vector.tensor_copy(ekdep[:], ek_sb[:, 0:1])
        zero_u = const_p.tile([128, 1], U32)
        nc.vector.tensor_scalar(zero_u[:], ekdep[:], 0, None, ALU.mult)
        rowd = []
        for kc in range(4):
            rt_ = const_p.tile([128, 1], U32, name=f"rowd_{kc}")
            nc.vector.tensor_tensor(out=rt_[:], in0=rowi[kc][:], in1=zero_u[:],
                                    op=ALU.add)
            rowd.append(rt_)
        et4_sb = const_p.tile([128, 4 * R], BF16)
        nc.gpsimd.indirect_dma_start(
            out=et4_sb[:], out_offset=None, in_=encT[:],
            in_offset=IndirectOffsetOnAxis(ap=rowd[0][:, :1], axis=0))
        et_sb = [et4_sb[:, kc * R:(kc + 1) * R] for kc in range(4)]
        enc_all = const_p.tile([128, RT * D], BF16)
        nc.gpsimd.indirect_dma_start(
            out=enc_all[:], out_offset=None, in_=enc_rows[:],
            in_offset=IndirectOffsetOnAxis(ap=rowd[0][:, :1], axis=0))
        w_sb = []
        for kc in range(4):
            wt_ = const_p.tile([128, V], BF16, name=f"w_sb_{kc}")
            nc.gpsimd.indirect_dma_start(
                out=wt_[:], out_offset=None, in_=w[:],
                in_offset=IndirectOffsetOnAxis(ap=rowd[kc][:, :1], axis=0))
            w_sb.append(wt_)

        xk_bufs = []
        for i_ in range(RT):
            xkt = const_p.tile([128, 128], BF16, name=f"xk_{i_}")
            nc.vector.memset(xkt[:], 0.0)
            xk_bufs.append(xkt)

        # PE warm-up: ramp the tensor-engine clock before the real work
        wmm = psb_p.tile([128, 128], F32, name="wmm", tag="bigb")
        for i_ in range(8):
            nc.tensor.matmul(wmm[:], ident[:], ident[:], start=(i_ == 0),
                             stop=(i_ == 7))
        wms = small_p.tile([128, 1], F32, name="wms")
        nc.vector.tensor_copy(wms[:], wmm[:, 0:1])

        def st_ap(kc, c0):
            if kc == 0:
                return st01_sb[:, c0:c0 + 128]
            if kc == 1:
                return st01_sb[0:128, R + c0:R + c0 + 128]
            return st2_sb[:, c0:c0 + 128]

        s_cols = cols_p.tile([128, RT, NVC], F32)
        c_cols = cols_p.tile([128, RT, NVC], F32)
        ltgt_cols = cols_p.tile([128, RT], F32)
        bias_cols = cols_p.tile([128, RT], F32)
        st_cols = cols_p.tile([128, RT], F32)
        cnt_cols = cols_p.tile([128, RT], F32)
        red_cols = cols_p.tile([128, 2 * RT], F32)
        nltgt_cols = cols_p.tile([128, RT], F32)
        thr_cols = cols_p.tile([128, RT], F32)
        dmax_cols = cols_p.tile([128, RT, NVC], F32)
        nc.vector.memset(c_cols[:], 0.0)
        nc.vector.memset(dmax_cols[:], -1e30)

        # ---------------- stage 0 -----------------------------------------
        # stats for all tiles; ONE batched Sqrt (single table load before
        # the exp set loads)
        mu_cols = cols_p.tile([128, RT], F32)
        vps_cols = cols_p.tile([128, RT], F32)
        rstd_cols = cols_p.tile([128, RT], F32)
        for rt in range(RT):
            stats = small_p.tile([128, 6], F32, name="stats")
            nc.vector.bn_stats(stats[:], stk_all[:, rt * SF:(rt + 1) * SF])
            mv = small_p.tile([128, 2], F32, name="mv")
            nc.vector.bn_aggr(mv[:], stats[:])
            nc.vector.tensor_copy(mu_cols[:, rt:rt + 1], mv[:, 0:1])
            nc.vector.tensor_scalar(vps_cols[:, rt:rt + 1], mv[:, 1:2],
                                    EPS_LN, None, ALU.add)
        nc.scalar.activation(rstd_cols[:], vps_cols[:], AF.Sqrt)
        nc.vector.reciprocal(rstd_cols[:], rstd_cols[:])
        for rt in range(RT):
            c0 = rt * 128
            rstd = rstd_cols[:, rt:rt + 1]

            psq = psa_p.tile([128, EDIM], F32, name="psq", tag="biga")
            for kc, (k0, k1) in enumerate([(0, 128), (128, 256), (256, 320)]):
                nc.tensor.matmul(psq[:], st_ap(kc, c0), pj[kc],
                                 start=(kc == 0), stop=(kc == 2))
            mu_c = small_p.tile([128, EDIM], F32, name="mu_c")
            nc.vector.tensor_scalar(mu_c[:], csum_b, mu_cols[:, rt:rt + 1],
                                    None, ALU.mult)
            x_t = small_p.tile([128, EDIM], F32, name="x_t")
            nc.vector.tensor_tensor(out=x_t[:], in0=psq[:], in1=mu_c[:],
                                    op=ALU.subtract)
            nc.vector.tensor_scalar(x_t[:], x_t[:], rstd, None, ALU.mult)

            pst = psa_p.tile([16, 128], F32, name="pst", tag="biga")
            nc.tensor.transpose(pst[:], x_t[:], ident[:])

            xk = xk_bufs[rt]
            xh_f = small_p.tile([16, 128], F32, name="xh_f")
            nc.vector.tensor_copy(xk[0:16, :], pst[:])           # hi (cast)
            nc.vector.tensor_copy(xh_f[:], xk[0:16, :])          # back to f32
            nc.vector.tensor_tensor(out=xh_f[:], in0=pst[:], in1=xh_f[:],
                                    op=ALU.subtract)             # residual
            nc.vector.tensor_copy(xk[32:48, :], xh_f[:])         # lo (cast)
            nc.vector.tensor_copy(xk[64:80, :], xk[0:16, :])
            nc.vector.tensor_copy(xk[96:112, :], xk[32:48, :])

        # ---------------- pipelined stage A(rt+1) / prep(rt) / B(rt) ----
        codes_i = [None] * RT
        codes_f = [None] * RT

        def emit_stage_a(rt):
            xk = xk_bufs[rt]
            mcat = scans_p.tile([128, 8 * NVC], F32, name="mcat")
            icat = scans_p.tile([128, 8 * NVC], U32, name="icat")
            for h in range(NVC):
                psd = psa_p.tile([128, VC], F32, name="psd", tag="biga")
                for j in range(VC // 512):
                    nc.tensor.matmul(
                        psd[:, j * 512:(j + 1) * 512], xk[:],
                        ek_sb[:, h * VC + j * 512:h * VC + (j + 1) * 512],
                        start=True, stop=True)
                nc.vector.max(mcat[:, 8 * h:8 * h + 8], psd[:])
                nc.vector.max_index(icat[:, 8 * h:8 * h + 8],
                                    mcat[:, 8 * h:8 * h + 8], psd[:])
            # cross-chunk combine: global max position p = 8*h_win in mcat
            ctx_hp = tc.high_priority()
            ctx_hp.__enter__()
            gm8 = small_p.tile([128, 8], F32, name="gm8")
            nc.vector.max(gm8[:], mcat[:])
            gp8 = small_p.tile([128, 8], U32, name="gp8")
            nc.vector.max_index(gp8[:], gm8[:], mcat[:])
            pf = small_p.tile([128, 1], F32, name="pf")
            nc.vector.tensor_copy(pf[:], gp8[:, 0:1])
            icf = small_p.tile([128, 8 * NVC], F32, name="icf")
            nc.vector.tensor_copy(icf[:], icat[:])
            # one-hot over chunks: eqv[:, h] = (pf == 8h)
            eqv = small_p.tile([128, NVC], F32, name="eqv")
            nc.vector.tensor_scalar(eqv[:], iota8[:, 0:NVC], pf[:], None,
                                    ALU.is_equal)
            sel = small_p.tile([128, NVC], F32, name="sel")
            nc.vector.tensor_tensor(out=sel[:], in0=eqv[:],
                                    in1=icf[:, 0:8 * NVC:8], op=ALU.mult)
            lsel = small_p.tile([128, 1], F32, name="lsel")
            nc.vector.reduce_sum(lsel[:], sel[:], axis=mybir.AxisListType.X)
            # code = VC*h + l = (VC/8)*p + l
            cf = small_p.tile([128, 1], F32, name=f"cf_{rt}", bufs=RT)
            nc.vector.scalar_tensor_tensor(
                out=cf[:], in0=pf[:], scalar=float(VC // 8), in1=lsel[:],
                op0=ALU.mult, op1=ALU.add)
            ci = small_p.tile([128, 1], U32, name=f"ci_{rt}", bufs=RT)
            nc.vector.tensor_copy(ci[:], cf[:])
            ctx_hp.__exit__(None, None, None)
            codes_i[rt] = ci
            codes_f[rt] = cf

        def emit_prep(rt):
            ctx_hp = tc.high_priority()
            ctx_hp.__enter__()
            g_t = scr_p.tile([128, D], F32, name="g_t")
            nc.gpsimd.indirect_dma_start(
                out=g_t[:], out_offset=None, in_=wt[:],
                in_offset=IndirectOffsetOnAxis(ap=codes_i[rt][:, :1], axis=0))
            prod = scr_p.tile([128, D], F32, name="prod")
            nc.vector.scalar_tensor_tensor(
                out=prod[:], in0=enc_all[:, rt * D:(rt + 1) * D], scalar=1.0,
                in1=g_t[:], op0=ALU.mult, op1=ALU.mult,
                accum_out=ltgt_cols[:, rt:rt + 1])
            # exp2 bias = -SHARP*(ltgt + MARGIN)
            nc.vector.tensor_scalar(bias_cols[:, rt:rt + 1],
                                    ltgt_cols[:, rt:rt + 1],
                                    -SHARP, -SHARP * MARGIN,
                                    ALU.mult, ALU.add)
            nc.vector.tensor_scalar(nltgt_cols[:, rt:rt + 1],
                                    ltgt_cols[:, rt:rt + 1], -1.0, None,
                                    ALU.mult)
            nc.vector.tensor_scalar(thr_cols[:, rt:rt + 1],
                                    ltgt_cols[:, rt:rt + 1], MARGIN, None,
                                    ALU.add)
            ctx_hp.__exit__(None, None, None)

        def emit_stage_b(rt):
            for h in range(NVC):
                psl = psb_p.tile([128, VC], F32, name="psl", tag="bigb")
                for kc in range(4):
                    for j in range(VC // 512):
                        nc.tensor.matmul(
                            psl[:, j * 512:(j + 1) * 512],
                            et4_sb[:, kc * R + rt * 128:kc * R + (rt + 1) * 128],
                            w_sb[kc][:, h * VC + j * 512:h * VC + (j + 1) * 512],
                            start=(kc == 0), stop=(kc == 3))
                exp_t = scr_p.tile([128, VC], BF16, name="exp_t")
                nc.scalar.activation(exp_t[:], psl[:], AF.Exp,
                                     bias=nltgt_cols[:, rt:rt + 1],
                                     accum_out=s_cols[:, rt, h:h + 1])
                di = h - (NVC - K_LIST[rt])
                if di >= 0:
                    # corr on DVE: chunk max via accumulate-max from PSUM
                    mx_t = scr_p.tile([128, VC], BF16, name="mx_t")
                    nc.vector.tensor_scalar(
                        mx_t[:], psl[:], 1.0, None, ALU.mult, ALU.max,
                        accum_out=dmax_cols[:, rt, di:di + 1])
                else:
                    # corr on ACT: sharp-exp count
                    shp_t = scr_p.tile([128, VC], BF16, name="shp_t")
                    nc.scalar.activation(shp_t[:], psl[:], AF.Exp,
                                         scale=SHARP,
                                         bias=bias_cols[:, rt:rt + 1],
                                         accum_out=c_cols[:, rt, h:h + 1])

        def emit_presence(rt):
            pidx_f = small_p.tile([128, 1], F32, name="pidx_f")
            nc.vector.tensor_tensor(out=pidx_f[:], in0=codes_f[rt][:],
                                    in1=mask_all[:, rt:rt + 1], op=ALU.mult)
            pidx = small_p.tile([128, 1], I32, name="pidx")
            nc.vector.tensor_copy(pidx[:], pidx_f[:])
            nc.gpsimd.indirect_dma_start(
                out=out_pres[:], out_offset=IndirectOffsetOnAxis(
                    ap=pidx[:, :1], axis=0),
                in_=ones_t[:, :], in_offset=None)

        emit_stage_a(0)
        emit_prep(0)
        emit_presence(0)
        emit_stage_a(1)
        emit_prep(1)
        emit_presence(1)
        for rt in range(RT):
            emit_stage_b(rt)
            if rt + 2 < RT:
                emit_stage_a(rt + 2)
                emit_prep(rt + 2)
                emit_presence(rt + 2)
            # per-tile partial reductions (DVE, schedule-anywhere)
            nc.vector.reduce_sum(st_cols[:, rt:rt + 1], s_cols[:, rt, :],
                                 axis=mybir.AxisListType.X)
            nc.vector.reduce_sum(cnt_cols[:, rt:rt + 1], c_cols[:, rt, :],
                                 axis=mybir.AxisListType.X)

        # ---------------- batched finalize -------------------------------
        lnS = small_p.tile([128, RT], F32, name="lnS")
        nc.scalar.activation(lnS[:], st_cols[:], AF.Ln)
        nc.vector.tensor_tensor(out=red_cols[:, 0:RT], in0=lnS[:],
                                in1=mask_all[:], op=ALU.mult)
        corr = small_p.tile([128, RT], F32, name="corr")
        nc.vector.tensor_scalar(corr[:], cnt_cols[:], 0.5, None, ALU.is_lt)
        dmx = small_p.tile([128, RT], F32, name="dmx")
        for rt in range(RT):
            nc.vector.tensor_reduce(op=ALU.max, out=dmx[:, rt:rt + 1],
                                    in_=dmax_cols[:, rt, :],
                                    axis=mybir.AxisListType.X)
        cor2 = small_p.tile([128, RT], F32, name="cor2")
        nc.vector.tensor_tensor(out=cor2[:], in0=dmx[:], in1=thr_cols[:],
                                op=ALU.is_le)
        nc.vector.tensor_tensor(out=corr[:], in0=corr[:], in1=cor2[:],
                                op=ALU.mult)
        nc.vector.tensor_tensor(out=red_cols[:, RT:2 * RT], in0=corr[:],
                                in1=mask_all[:], op=ALU.mult)

        # ---------------- partition reduction ---------------------------
        psr = psa_p.tile([1, 2 * RT], F32, name="psr", tag="biga")
        nc.tensor.matmul(psr[:], ones_t[:], red_cols[:], start=True, stop=True)
        fin = small_p.tile([1, 8], F32, name="fin")
        nc.vector.reduce_sum(fin[:, 0:1], psr[0:1, 0:RT], axis=mybir.AxisListType.X)
        nc.vector.reduce_sum(fin[:, 1:2], psr[0:1, RT:2 * RT],
                             axis=mybir.AxisListType.X)
        nc.vector.memset(fin[:, 2:8], 0.0)
        nc.sync.dma_start(out_stats[:], fin[:])

    nc.compile()
    _NC_CACHE["nc"] = nc
    return nc


def _row_map(lengths):
    """Valid CE rows (b, j): enc frame j, target frame j+1; j+1 <= L_b-1."""
    L = np.asarray(lengths).astype(np.int64) // STRIDE
    bs, js = [], []
    for b in range(B):
        n = int(L[b]) - 1
        bs.extend([b] * n)
        js.extend(range(n))
    nvalid = len(bs)
    pad = NCORES * R - nvalid
    assert pad >= 0, f"too many valid rows: {nvalid}"
    bs = np.array(bs + [0] * pad, dtype=np.int64)
    js = np.array(js + [0] * pad, dtype=np.int64)
    vm = np.zeros(NCORES * R, dtype=bool)
    vm[:nvalid] = True
    return bs, js, vm, nvalid


def _prep_core_inputs(inputs, core, row_map):
    feats = np.asarray(inputs["feats"])
    enc = np.asarray(inputs["encoder_out"])
    bs, js, vm, _ = row_map
    sl = slice(core * R, (core + 1) * R)
    b_c, j_c, v_c = bs[sl], js[sl], vm[sl]

    fb = feats.reshape(B, N, SF)
    stack_rows = fb[b_c, j_c + 1].astype(np.float32)
    stack_rows[~v_c] = 0.0
    enc_r = enc[b_c, j_c].astype(np.float32)
    enc_r[~v_c] = 0.0
    maskce = v_c.astype(np.float32).reshape(R, 1)

    def tilemajor(a, nt):
        # [nt*128, F] -> [128, nt*F]: row i*128+p maps to [p, i*F:(i+1)*F]
        f = a.shape[1]
        return np.ascontiguousarray(
            a.reshape(nt, 128, f).transpose(1, 0, 2).reshape(128, nt * f))

    stk_pack = np.concatenate(
        [tilemajor(stack_rows, RT), tilemajor(maskce.reshape(R, 1), RT)],
        axis=1)
    sT = stack_rows.T  # [320, R]
    st01p = np.concatenate([sT[0:128], sT[128:256]], axis=1)  # [128, 2R]
    encT = enc_r.T.astype(ml_dtypes.bfloat16)  # [512, R]
    et4 = tilemajor(encT, 4)  # [128, 4R]

    return {
        "stack_rows": np.ascontiguousarray(stk_pack),
        "st01": np.ascontiguousarray(st01p),
        "st2": np.ascontiguousarray(sT[256:320]),
        "encT": np.ascontiguousarray(et4),
        "enc_rows": tilemajor(enc_r.astype(ml_dtypes.bfloat16), RT),
    }


def _prep_shared_inputs(inputs):
    proj = np.asarray(inputs["projection"], dtype=np.float32)
    emb = np.asarray(inputs["embeddings"], dtype=np.float32)
    top = np.asarray(inputs["top_n_out"], dtype=np.float32)

    pjp = np.zeros((128, 4 * EDIM), np.float32)
    pjp[:, 0:EDIM] = proj[0:128]
    pjp[:, EDIM:2 * EDIM] = proj[128:256]
    pjp[0:64, 2 * EDIM:3 * EDIM] = proj[256:320]
    pjp[:, 3 * EDIM:4 * EDIM] = proj.sum(0, keepdims=True)

    Et = np.ascontiguousarray(emb[:, 0, :].T, dtype=np.float32)  # [16, V]
    Eh = Et.astype(ml_dtypes.bfloat16).astype(np.float32)
    El = (Et - Eh).astype(ml_dtypes.bfloat16).astype(np.float32)
    # dense [64,V]; device expands to quadrants [Eh,Eh,El,El] pairing
    # xk quadrants [xh,xl,xh,xl]: hh + lh + hl + ll
    Ek = np.concatenate(
        [Eh, Eh, El, El], axis=0).astype(ml_dtypes.bfloat16)

    W = np.ascontiguousarray(top[0, 0], dtype=np.float32)        # [D, V]
    Wt = np.ascontiguousarray(W.T)                               # [V, D]
    return {
        "pjp": pjp,
        "Ek": np.ascontiguousarray(Ek),
        "W": np.ascontiguousarray(W.astype(ml_dtypes.bfloat16)),
        "Wt": Wt,
    }


def _combine(results, inputs, row_map):
    _, _, _, nvalid = row_map
    num_codes = float(nvalid)

    nll_sum = 0.0
    corr_sum = 0.0
    pres = np.zeros(V, dtype=bool)
    for r in results:
        st = np.asarray(r["out_stats"]).reshape(-1)
        nll_sum += float(st[0])
        corr_sum += float(st[1])
        pres |= np.asarray(r["out_pres"]).reshape(-1) > 0.0
    # reference scatters index 0 for every masked grid row; those exist
    # whenever num_codes < B*(N-1) (always here)
    if nvalid < B * (N - 1):
        pres[0] = True

    loss = np.float32(nll_sum / num_codes)
    acc = np.float32(corr_sum / num_codes)
    uniq = np.float32(pres.sum())
    return np.array([loss, acc, np.float32(num_codes), uniq], dtype=np.float32)


def _run(inputs, trace=False):
    from concourse.bass_utils import run_bass_kernel_spmd
    nc = _build_program()
    row_map = _row_map(inputs["feats_lengths"])
    shared = _prep_shared_inputs(inputs)
    in_maps = []
    for core in range(NCORES):
        m = dict(shared)
        m.update(_prep_core_inputs(inputs, core, row_map))
        in_maps.append(m)
    res = run_bass_kernel_spmd(nc, in_maps, core_ids=list(range(NCORES)),
                               trace=trace)
    out = _combine(res.results, inputs, row_map)
    return out, res


def _run_sim(inputs, core=0):
    """Single-core simulator run (correctness debugging)."""
    from concourse.bass_interp import CoreSim
    nc = _build_program()
    row_map = _row_map(inputs["feats_lengths"])
    m = dict(_prep_shared_inputs(inputs))
    m.update(_prep_core_inputs(inputs, core, row_map))
    sim = CoreSim(nc, require_finite=False, require_nnan=False)
    for k, v in m.items():
        sim.tensor(k)[:] = v
    sim.simulate()
    return {k: np.array(sim.tensor(k)) for k in ("out_stats", "out_pres")}


def kernel(**inputs) -> np.ndarray:
    out, _ = _run(inputs, trace=False)
    return out


# revision 26
# speedup vs baseline: 1.0597x; 1.0597x over previous
"""Trainium2 Bass kernel for nn_NestRQModel (NEST-RQ pretraining loss).

Strategy: data-parallel over COMPACTED valid CE rows.  The reference only
counts rows (b, j) with j+1 < lengths[b]//4 (5700 of 8176); the host builds
that row list from feats_lengths (pure reindexing) and pads to 6144 = 8
cores x 768 rows.  Each core computes partial masked sums (nll, corr) and a
vocab-presence scatter; the host combines into the 4 scalar outputs.

Per-core pipeline (768 rows = 6 row tiles):
  stage 0: LayerNorm stats (bn_stats); projection matmul with LN folded in
           afterwards as a per-row affine fix (LN commutes through the
           linear map); rstd = exp(-0.5*ln(var+eps)) on ACT, with all Ln's
           batched before all Exp's (one table switch, no Sqrt set);
           x packed to bf16 hi/lo Kahan rows for stage A.
  stage A: dots = x @ E^T as K=128 Kahan bf16 matmuls (error ~1e-6, full
           PE rate); argmax runs DIRECTLY ON PSUM: per 1024-chunk
           max8/find_index8, then a vectorized cross-chunk combine.
  stage B: logits = enc @ W bf16 (full PE rate); ACT Exp with accum_out
           -> row sum-of-exp; corr via a SECOND sharp exp on ACT:
           sum_v exp(S*(logit_v - thr)) < 0.5  <=>  no logit above
           thr = ltgt + margin  (overflow -> inf -> counted incorrect,
           which is the right answer);  tgt logit = indirect-DMA gather
           of W^T rows + fused dot (scalar_tensor_tensor accum).
  PSUM is split: stage-A chunks and stage-B chunks each get their own
  2-buffer [128,1024] pool so DVE scans never block PE/ACT progress.
  presence: indirect-DMA scatter of 1.0 at (code*mask) indices.
"""
import os
import sys

import numpy as np

os.environ.setdefault("MYCRO_LOCAL_CACHE", "1")

try:
    import concourse.bass as bass
except ImportError:
    sys.path.insert(0, "/opt/trn_rl_repo")
    import concourse.bass as bass

import ml_dtypes
import concourse.bacc as bacc
import concourse.tile as tile
from concourse import mybir
from concourse.bass import IndirectOffsetOnAxis
from concourse.masks import make_identity
from contextlib import ExitStack

F32 = mybir.dt.float32
BF16 = mybir.dt.bfloat16
U32 = mybir.dt.uint32
I32 = mybir.dt.int32
AF = mybir.ActivationFunctionType
ALU = mybir.AluOpType

# problem constants
NCORES = 8
B, T, F = 16, 2048, 80
STK, STRIDE = 4, 4
N = 512                 # frames per batch after subsampling
SF = STK * F            # 320 stacked feature dim
EDIM = 16
V = 8192
D = 512                 # encoder dim
R = 768                 # compacted rows per core (8*768 = 6144 >= 5700)
RT = R // 128           # 6 row tiles
VC = 1024               # vocab/psum chunk (2 PSUM banks)
NVC = V // VC           # 8
EPS_LN = 1e-6
MARGIN = 7e-3           # corr margin: covers bf16 matmul noise (~1.5e-3)
SHARP = 400.0           # corr sharp-exp scale
K_LIST = [int(x) for x in os.environ.get(
    "NESTRQ_KLIST", "0,0,0,0,3,6").split(",")]  # corr chunks on DVE/tile

_NC_CACHE = {}


def _build_program():
    if "nc" in _NC_CACHE:
        return _NC_CACHE["nc"]
    nc = bacc.Bacc("TRN2", target_bir_lowering=False)

    # stk_all = [stacked feats | maskce] packed; pjp = [pj0|pj1|pj2pad|csum]
    stack_rows = nc.declare_dram_parameter("stack_rows", [128, RT * SF + RT],
                                           F32, isOutput=False)
    st01 = nc.declare_dram_parameter("st01", [128, 2 * R], F32, isOutput=False)
    st2 = nc.declare_dram_parameter("st2", [64, R], F32, isOutput=False)
    pjp = nc.declare_dram_parameter("pjp", [128, 4 * EDIM], F32, isOutput=False)
    ek = nc.declare_dram_parameter("Ek", [64, V], BF16, isOutput=False)
    w = nc.declare_dram_parameter("W", [D, V], BF16, isOutput=False)
    wt = nc.declare_dram_parameter("Wt", [V, D], F32, isOutput=False)
    encT = nc.declare_dram_parameter("encT", [128, 4 * R], BF16, isOutput=False)
    enc_rows = nc.declare_dram_parameter("enc_rows", [128, RT * D], BF16, isOutput=False)

    out_stats = nc.declare_dram_parameter("out_stats", [1, 8], F32, isOutput=True)
    out_pres = nc.declare_dram_parameter("out_pres", [V, 1], F32, isOutput=True)

    with tile.TileContext(nc) as tc, ExitStack() as ctx:
        const_p = ctx.enter_context(tc.tile_pool(name="const", bufs=1))
        small_p = ctx.enter_context(tc.tile_pool(name="small", bufs=4))
        cols_p = ctx.enter_context(tc.tile_pool(name="cols", bufs=1))
        stage0_p = ctx.enter_context(tc.tile_pool(name="stage0", bufs=2))
        scans_p = ctx.enter_context(tc.tile_pool(name="scans", bufs=3))
        scr_p = ctx.enter_context(tc.tile_pool(name="scr", bufs=2))
        psa_p = ctx.enter_context(tc.tile_pool(name="psa", bufs=2, space="PSUM"))
        psb_p = ctx.enter_context(tc.tile_pool(name="psb", bufs=2, space="PSUM"))

        # stage-0 path: small direct DMAs split across sync/scalar queues
        stk_all = const_p.tile([128, RT * SF + RT], F32)
        nc.sync.dma_start(stk_all[:], stack_rows[:])
        st01_sb = const_p.tile([128, 2 * R], F32)
        nc.scalar.dma_start(st01_sb[:], st01[:])
        st2_sb = const_p.tile([64, R], F32)
        nc.scalar.dma_start(st2_sb[:], st2[:])
        pjp_sb = const_p.tile([128, 4 * EDIM], F32)
        nc.sync.dma_start(pjp_sb[:], pjp[:])
        st_sb = [st01_sb[:, 0:R], st01_sb[0:128, R:2 * R], st2_sb[:]]
        pj = [pjp_sb[:, 0:EDIM], pjp_sb[:, EDIM:2 * EDIM],
              pjp_sb[0:64, 2 * EDIM:3 * EDIM]]
        csum_b = pjp_sb[:, 3 * EDIM:4 * EDIM]
        mask_all = stk_all[:, RT * SF:RT * SF + RT]

        ident = const_p.tile([128, 128], F32)
        make_identity(nc, ident[:])
        eps_t = const_p.tile([128, 1], F32)
        nc.vector.memset(eps_t[:], EPS_LN)
        ones_t = const_p.tile([128, 1], F32)
        nc.vector.memset(ones_t[:], 1.0)
        # iota8x8 = [0, 8, 16, ..., 56] per partition
        iota8 = const_p.tile([128, 8], F32)
        for h in range(8):
            nc.vector.memset(iota8[:, h:h + 1], float(8 * h))

        # heavy loads via SWDGE indirect gathers (fast descriptor gen)
        rowi = []
        for kc in range(4):
            it = const_p.tile([128, 1], U32, name=f"rowi_{kc}")
            nc.gpsimd.iota(it[:], pattern=[[0, 1]], base=128 * kc,
                           channel_multiplier=1)
            rowi.append(it)

        # Ek transferred dense [64,V] = [Eh|El|Eh|El]; expanded into the
        # 4 SBUF quadrants (junk rows pair with xk's zero rows)
        ek_sb = const_p.tile([128, V], BF16)
        iota32 = const_p.tile([32, 1], U32)
        nc.gpsimd.iota(iota32[:], pattern=[[0, 1]], base=0,
                       channel_multiplier=1)
        m16 = const_p.tile([32, 1], U32)
        nc.vector.tensor_scalar(m16[:], iota32[:], 16, None, ALU.is_lt)
        ekoff = []
        for q in range(4):
            ot = const_p.tile([32, 1], U32, name=f"ekoff_{q}")
            nc.vector.tensor_scalar(ot[:], iota32[:], 16 * q, None, ALU.add)
            nc.vector.tensor_tensor(out=ot[:], in0=ot[:], in1=m16[:],
                                    op=ALU.mult)
            ekoff.append(ot)
        for q in range(4):
            nc.gpsimd.indirect_dma_start(
                out=ek_sb[32 * q:32 * (q + 1), :], out_offset=None, in_=ek[:],
                in_offset=IndirectOffsetOnAxis(ap=ekoff[q][:, :1], axis=0))
        et4_sb = const_p.tile([128, 4 * R], BF16)
        nc.gpsimd.indirect_dma_start(
            out=et4_sb[:], out_offset=None, in_=encT[:],
            in_offset=IndirectOffsetOnAxis(ap=rowi[0][:, :1], axis=0))
        et_sb = [et4_sb[:, kc * R:(kc + 1) * R] for kc in range(4)]
        w_sb = []
        for kc in range(4):
            wt_ = const_p.tile([128, V], BF16, name=f"w_sb_{kc}")
            nc.gpsimd.indirect_dma_start(
                out=wt_[:], out_offset=None, in_=w[:],
                in_offset=IndirectOffsetOnAxis(ap=rowi[kc][:, :1], axis=0))
            w_sb.append(wt_)
        enc_all = const_p.tile([128, RT * D], BF16)
        nc.gpsimd.indirect_dma_start(
            out=enc_all[:], out_offset=None, in_=enc_rows[:],
            in_offset=IndirectOffsetOnAxis(ap=rowi[0][:, :1], axis=0))
        xk_bufs = []
        for i_ in range(RT):
            xkt = const_p.tile([128, 128], BF16, name=f"xk_{i_}")
            nc.vector.memset(xkt[:], 0.0)
            xk_bufs.append(xkt)

        # PE warm-up: ramp the tensor-engine clock before the real work
        wmm = psb_p.tile([128, 128], F32, name="wmm", tag="bigb")
        for i_ in range(8):
            nc.tensor.matmul(wmm[:], ident[:], ident[:], start=(i_ == 0),
                             stop=(i_ == 7))
        wms = small_p.tile([128, 1], F32, name="wms")
        nc.vector.tensor_copy(wms[:], wmm[:, 0:1])

        def st_ap(kc, c0):
            if kc == 0:
                return st01_sb[:, c0:c0 + 128]
            if kc == 1:
                return st01_sb[0:128, R + c0:R + c0 + 128]
            return st2_sb[:, c0:c0 + 128]

        s_cols = cols_p.tile([128, RT, NVC], F32)
        c_cols = cols_p.tile([128, RT, NVC], F32)
        ltgt_cols = cols_p.tile([128, RT], F32)
        bias_cols = cols_p.tile([128, RT], F32)
        st_cols = cols_p.tile([128, RT], F32)
        cnt_cols = cols_p.tile([128, RT], F32)
        red_cols = cols_p.tile([128, 2 * RT], F32)
        nltgt_cols = cols_p.tile([128, RT], F32)
        thr_cols = cols_p.tile([128, RT], F32)
        dmax_cols = cols_p.tile([128, RT, NVC], F32)
        nc.vector.memset(c_cols[:], 0.0)
        nc.vector.memset(dmax_cols[:], -1e30)

        # ---------------- stage 0 -----------------------------------------
        # stats for all tiles; ONE batched Sqrt (single table load before
        # the exp set loads)
        mu_cols = cols_p.tile([128, RT], F32)
        vps_cols = cols_p.tile([128, RT], F32)
        rstd_cols = cols_p.tile([128, RT], F32)
        for rt in range(RT):
            stats = small_p.tile([128, 6], F32, name="stats")
            nc.vector.bn_stats(stats[:], stk_all[:, rt * SF:(rt + 1) * SF])
            mv = small_p.tile([128, 2], F32, name="mv")
            nc.vector.bn_aggr(mv[:], stats[:])
            nc.vector.tensor_copy(mu_cols[:, rt:rt + 1], mv[:, 0:1])
            nc.vector.tensor_scalar(vps_cols[:, rt:rt + 1], mv[:, 1:2],
                                    EPS_LN, None, ALU.add)
        nc.scalar.activation(rstd_cols[:], vps_cols[:], AF.Sqrt)
        nc.vector.reciprocal(rstd_cols[:], rstd_cols[:])
        for rt in range(RT):
            c0 = rt * 128
            rstd = rstd_cols[:, rt:rt + 1]

            psq = psa_p.tile([128, EDIM], F32, name="psq", tag="biga")
            for kc, (k0, k1) in enumerate([(0, 128), (128, 256), (256, 320)]):
                nc.tensor.matmul(psq[:], st_ap(kc, c0), pj[kc],
                                 start=(kc == 0), stop=(kc == 2))
            mu_c = small_p.tile([128, EDIM], F32, name="mu_c")
            nc.vector.tensor_scalar(mu_c[:], csum_b, mu_cols[:, rt:rt + 1],
                                    None, ALU.mult)
            x_t = small_p.tile([128, EDIM], F32, name="x_t")
            nc.vector.tensor_tensor(out=x_t[:], in0=psq[:], in1=mu_c[:],
                                    op=ALU.subtract)
            nc.vector.tensor_scalar(x_t[:], x_t[:], rstd, None, ALU.mult)

            pst = psa_p.tile([16, 128], F32, name="pst", tag="biga")
            nc.tensor.transpose(pst[:], x_t[:], ident[:])

            xk = xk_bufs[rt]
            xh_f = small_p.tile([16, 128], F32, name="xh_f")
            nc.vector.tensor_copy(xk[0:16, :], pst[:])           # hi (cast)
            nc.vector.tensor_copy(xh_f[:], xk[0:16, :])          # back to f32
            nc.vector.tensor_tensor(out=xh_f[:], in0=pst[:], in1=xh_f[:],
                                    op=ALU.subtract)             # residual
            nc.vector.tensor_copy(xk[32:48, :], xh_f[:])         # lo (cast)
            nc.vector.tensor_copy(xk[64:80, :], xk[0:16, :])
            nc.vector.tensor_copy(xk[96:112, :], xk[32:48, :])

        # ---------------- pipelined stage A(rt+1) / prep(rt) / B(rt) ----
        codes_i = [None] * RT
        codes_f = [None] * RT

        def emit_stage_a(rt):
            xk = xk_bufs[rt]
            mcat = scans_p.tile([128, 8 * NVC], F32, name="mcat")
            icat = scans_p.tile([128, 8 * NVC], U32, name="icat")
            for h in range(NVC):
                psd = psa_p.tile([128, VC], F32, name="psd", tag="biga")
                for j in range(VC // 512):
                    nc.tensor.matmul(
                        psd[:, j * 512:(j + 1) * 512], xk[:],
                        ek_sb[:, h * VC + j * 512:h * VC + (j + 1) * 512],
                        start=True, stop=True)
                nc.vector.max(mcat[:, 8 * h:8 * h + 8], psd[:])
                nc.vector.max_index(icat[:, 8 * h:8 * h + 8],
                                    mcat[:, 8 * h:8 * h + 8], psd[:])
            # cross-chunk combine: global max position p = 8*h_win in mcat
            ctx_hp = tc.high_priority()
            ctx_hp.__enter__()
            gm8 = small_p.tile([128, 8], F32, name="gm8")
            nc.vector.max(gm8[:], mcat[:])
            gp8 = small_p.tile([128, 8], U32, name="gp8")
            nc.vector.max_index(gp8[:], gm8[:], mcat[:])
            pf = small_p.tile([128, 1], F32, name="pf")
            nc.vector.tensor_copy(pf[:], gp8[:, 0:1])
            icf = small_p.tile([128, 8 * NVC], F32, name="icf")
            nc.vector.tensor_copy(icf[:], icat[:])
            # one-hot over chunks: eqv[:, h] = (pf == 8h)
            eqv = small_p.tile([128, NVC], F32, name="eqv")
            nc.vector.tensor_scalar(eqv[:], iota8[:, 0:NVC], pf[:], None,
                                    ALU.is_equal)
            sel = small_p.tile([128, NVC], F32, name="sel")
            nc.vector.tensor_tensor(out=sel[:], in0=eqv[:],
                                    in1=icf[:, 0:8 * NVC:8], op=ALU.mult)
            lsel = small_p.tile([128, 1], F32, name="lsel")
            nc.vector.reduce_sum(lsel[:], sel[:], axis=mybir.AxisListType.X)
            # code = VC*h + l = (VC/8)*p + l
            cf = small_p.tile([128, 1], F32, name=f"cf_{rt}", bufs=RT)
            nc.vector.scalar_tensor_tensor(
                out=cf[:], in0=pf[:], scalar=float(VC // 8), in1=lsel[:],
                op0=ALU.mult, op1=ALU.add)
            ci = small_p.tile([128, 1], U32, name=f"ci_{rt}", bufs=RT)
            nc.vector.tensor_copy(ci[:], cf[:])
            ctx_hp.__exit__(None, None, None)
            codes_i[rt] = ci
            codes_f[rt] = cf

        def emit_prep(rt):
            ctx_hp = tc.high_priority()
            ctx_hp.__enter__()
            g_t = scr_p.tile([128, D], F32, name="g_t")
            nc.gpsimd.indirect_dma_start(
                out=g_t[:], out_offset=None, in_=wt[:],
                in_offset=IndirectOffsetOnAxis(ap=codes_i[rt][:, :1], axis=0))
            prod = scr_p.tile([128, D], F32, name="prod")
            nc.vector.scalar_tensor_tensor(
                out=prod[:], in0=enc_all[:, rt * D:(rt + 1) * D], scalar=1.0,
                in1=g_t[:], op0=ALU.mult, op1=ALU.mult,
                accum_out=ltgt_cols[:, rt:rt + 1])
            # exp2 bias = -SHARP*(ltgt + MARGIN)
            nc.vector.tensor_scalar(bias_cols[:, rt:rt + 1],
                                    ltgt_cols[:, rt:rt + 1],
                                    -SHARP, -SHARP * MARGIN,
                                    ALU.mult, ALU.add)
            nc.vector.tensor_scalar(nltgt_cols[:, rt:rt + 1],
                                    ltgt_cols[:, rt:rt + 1], -1.0, None,
                                    ALU.mult)
            nc.vector.tensor_scalar(thr_cols[:, rt:rt + 1],
                                    ltgt_cols[:, rt:rt + 1], MARGIN, None,
                                    ALU.add)
            ctx_hp.__exit__(None, None, None)

        def emit_stage_b(rt):
            for h in range(NVC):
                psl = psb_p.tile([128, VC], F32, name="psl", tag="bigb")
                for kc in range(4):
                    for j in range(VC // 512):
                        nc.tensor.matmul(
                            psl[:, j * 512:(j + 1) * 512],
                            et4_sb[:, kc * R + rt * 128:kc * R + (rt + 1) * 128],
                            w_sb[kc][:, h * VC + j * 512:h * VC + (j + 1) * 512],
                            start=(kc == 0), stop=(kc == 3))
                exp_t = scr_p.tile([128, VC], BF16, name="exp_t")
                nc.scalar.activation(exp_t[:], psl[:], AF.Exp,
                                     bias=nltgt_cols[:, rt:rt + 1],
                                     accum_out=s_cols[:, rt, h:h + 1])
                di = h - (NVC - K_LIST[rt])
                if di >= 0:
                    # corr on DVE: chunk max via accumulate-max from PSUM
                    mx_t = scr_p.tile([128, VC], BF16, name="mx_t")
                    nc.vector.tensor_scalar(
                        mx_t[:], psl[:], 1.0, None, ALU.mult, ALU.max,
                        accum_out=dmax_cols[:, rt, di:di + 1])
                else:
                    # corr on ACT: sharp-exp count
                    shp_t = scr_p.tile([128, VC], BF16, name="shp_t")
                    nc.scalar.activation(shp_t[:], psl[:], AF.Exp,
                                         scale=SHARP,
                                         bias=bias_cols[:, rt:rt + 1],
                                         accum_out=c_cols[:, rt, h:h + 1])

        def emit_presence(rt):
            pidx_f = small_p.tile([128, 1], F32, name="pidx_f")
            nc.vector.tensor_tensor(out=pidx_f[:], in0=codes_f[rt][:],
                                    in1=mask_all[:, rt:rt + 1], op=ALU.mult)
            pidx = small_p.tile([128, 1], I32, name="pidx")
            nc.vector.tensor_copy(pidx[:], pidx_f[:])
            nc.gpsimd.indirect_dma_start(
                out=out_pres[:], out_offset=IndirectOffsetOnAxis(
                    ap=pidx[:, :1], axis=0),
                in_=ones_t[:, :], in_offset=None)

        emit_stage_a(0)
        emit_prep(0)
        emit_presence(0)
        emit_stage_a(1)
        emit_prep(1)
        emit_presence(1)
        for rt in range(RT):
            emit_stage_b(rt)
            if rt + 2 < RT:
                emit_stage_a(rt + 2)
                emit_prep(rt + 2)
                emit_presence(rt + 2)
            # per-tile partial reductions (DVE, schedule-anywhere)
            nc.vector.reduce_sum(st_cols[:, rt:rt + 1], s_cols[:, rt, :],
                                 axis=mybir.AxisListType.X)
            nc.vector.reduce_sum(cnt_cols[:, rt:rt + 1], c_cols[:, rt, :],
                                 axis=mybir.AxisListType.X)

        # ---------------- batched finalize -------------------------------
        lnS = small_p.tile([128, RT], F32, name="lnS")
        nc.scalar.activation(lnS[:], st_cols[:], AF.Ln)
        nc.vector.tensor_tensor(out=red_cols[:, 0:RT], in0=lnS[:],
                                in1=mask_all[:], op=ALU.mult)
        corr = small_p.tile([128, RT], F32, name="corr")
        nc.vector.tensor_scalar(corr[:], cnt_cols[:], 0.5, None, ALU.is_lt)
        dmx = small_p.tile([128, RT], F32, name="dmx")
        for rt in range(RT):
            nc.vector.tensor_reduce(op=ALU.max, out=dmx[:, rt:rt + 1],
                                    in_=dmax_cols[:, rt, :],
                                    axis=mybir.AxisListType.X)
        cor2 = small_p.tile([128, RT], F32, name="cor2")
        nc.vector.tensor_tensor(out=cor2[:], in0=dmx[:], in1=thr_cols[:],
                                op=ALU.is_le)
        nc.vector.tensor_tensor(out=corr[:], in0=corr[:], in1=cor2[:],
                                op=ALU.mult)
        nc.vector.tensor_tensor(out=red_cols[:, RT:2 * RT], in0=corr[:],
                                in1=mask_all[:], op=ALU.mult)

        # ---------------- partition reduction ---------------------------
        psr = psa_p.tile([1, 2 * RT], F32, name="psr", tag="biga")
        nc.tensor.matmul(psr[:], ones_t[:], red_cols[:], start=True, stop=True)
        fin = small_p.tile([1, 8], F32, name="fin")
        nc.vector.reduce_sum(fin[:, 0:1], psr[0:1, 0:RT], axis=mybir.AxisListType.X)
        nc.vector.reduce_sum(fin[:, 1:2], psr[0:1, RT:2 * RT],
                             axis=mybir.AxisListType.X)
        nc.vector.memset(fin[:, 2:8], 0.0)
        nc.sync.dma_start(out_stats[:], fin[:])

    nc.compile()
    _NC_CACHE["nc"] = nc
    return nc


def _row_map(lengths):
    """Valid CE rows (b, j): enc frame j, target frame j+1; j+1 <= L_b-1."""
    L = np.asarray(lengths).astype(np.int64) // STRIDE
    bs, js = [], []
    for b in range(B):
        n = int(L[b]) - 1
        bs.extend([b] * n)
        js.extend(range(n))
    nvalid = len(bs)
    pad = NCORES * R - nvalid
    assert pad >= 0, f"too many valid rows: {nvalid}"
    bs = np.array(bs + [0] * pad, dtype=np.int64)
    js = np.array(js + [0] * pad, dtype=np.int64)
    vm = np.zeros(NCORES * R, dtype=bool)
    vm[:nvalid] = True
    return bs, js, vm, nvalid


def _prep_core_inputs(inputs, core, row_map):
    feats = np.asarray(inputs["feats"])
    enc = np.asarray(inputs["encoder_out"])
    bs, js, vm, _ = row_map
    sl = slice(core * R, (core + 1) * R)
    b_c, j_c, v_c = bs[sl], js[sl], vm[sl]

    fb = feats.reshape(B, N, SF)
    stack_rows = fb[b_c, j_c + 1].astype(np.float32)
    stack_rows[~v_c] = 0.0
    enc_r = enc[b_c, j_c].astype(np.float32)
    enc_r[~v_c] = 0.0
    maskce = v_c.astype(np.float32).reshape(R, 1)

    def tilemajor(a, nt):
        # [nt*128, F] -> [128, nt*F]: row i*128+p maps to [p, i*F:(i+1)*F]
        f = a.shape[1]
        return np.ascontiguousarray(
            a.reshape(nt, 128, f).transpose(1, 0, 2).reshape(128, nt * f))

    stk_pack = np.concatenate(
        [tilemajor(stack_rows, RT), tilemajor(maskce.reshape(R, 1), RT)],
        axis=1)
    sT = stack_rows.T  # [320, R]
    st01p = np.concatenate([sT[0:128], sT[128:256]], axis=1)  # [128, 2R]
    encT = enc_r.T.astype(ml_dtypes.bfloat16)  # [512, R]
    et4 = tilemajor(encT, 4)  # [128, 4R]

    return {
        "stack_rows": np.ascontiguousarray(stk_pack),
        "st01": np.ascontiguousarray(st01p),
        "st2": np.ascontiguousarray(sT[256:320]),
        "encT": np.ascontiguousarray(et4),
        "enc_rows": tilemajor(enc_r.astype(ml_dtypes.bfloat16), RT),
    }


def _prep_shared_inputs(inputs):
    proj = np.asarray(inputs["projection"], dtype=np.float32)
    emb = np.asarray(inputs["embeddings"], dtype=np.float32)
    top = np.asarray(inputs["top_n_out"], dtype=np.float32)

    pjp = np.zeros((128, 4 * EDIM), np.float32)
    pjp[:, 0:EDIM] = proj[0:128]
    pjp[:, EDIM:2 * EDIM] = proj[128:256]
    pjp[0:64, 2 * EDIM:3 * EDIM] = proj[256:320]
    pjp[:, 3 * EDIM:4 * EDIM] = proj.sum(0, keepdims=True)

    Et = np.ascontiguousarray(emb[:, 0, :].T, dtype=np.float32)  # [16, V]
    Eh = Et.astype(ml_dtypes.bfloat16).astype(np.float32)
    El = (Et - Eh).astype(ml_dtypes.bfloat16).astype(np.float32)
    # dense [64,V]; device expands to quadrants [Eh,Eh,El,El] pairing
    # xk quadrants [xh,xl,xh,xl]: hh + lh + hl + ll
    Ek = np.concatenate(
        [Eh, Eh, El, El], axis=0).astype(ml_dtypes.bfloat16)

    W = np.ascontiguousarray(top[0, 0], dtype=np.float32)        # [D, V]
    Wt = np.ascontiguousarray(W.T)                               # [V, D]
    return {
        "pjp": pjp,
        "Ek": np.ascontiguousarray(Ek),
        "W": np.ascontiguousarray(W.astype(ml_dtypes.bfloat16)),
        "Wt": Wt,
    }


def _combine(results, inputs, row_map):
    _, _, _, nvalid = row_map
    num_codes = float(nvalid)

    nll_sum = 0.0
    corr_sum = 0.0
    pres = np.zeros(V, dtype=bool)
    for r in results:
        st = np.asarray(r["out_stats"]).reshape(-1)
        nll_sum += float(st[0])
        corr_sum += float(st[1])
        pres |= np.asarray(r["out_pres"]).reshape(-1) > 0.0
    # reference scatters index 0 for every masked grid row; those exist
    # whenever num_codes < B*(N-1) (always here)
    if nvalid < B * (N - 1):
        pres[0] = True

    loss = np.float32(nll_sum / num_codes)
    acc = np.float32(corr_sum / num_codes)
    uniq = np.float32(pres.sum())
    return np.array([loss, acc, np.float32(num_codes), uniq], dtype=np.float32)


def _run(inputs, trace=False):
    from concourse.bass_utils import run_bass_kernel_spmd
    nc = _build_program()
    row_map = _row_map(inputs["feats_lengths"])
    shared = _prep_shared_inputs(inputs)
    in_maps = []
    for core in range(NCORES):
        m = dict(shared)
        m.update(_prep_core_inputs(inputs, core, row_map))
        in_maps.append(m)
    res = run_bass_kernel_spmd(nc, in_maps, core_ids=list(range(NCORES)),
                               trace=trace)
    out = _combine(res.results, inputs, row_map)
    return out, res


def _run_sim(inputs, core=0):
    """Single-core simulator run (correctness debugging)."""
    from concourse.bass_interp import CoreSim
    nc = _build_program()
    row_map = _row_map(inputs["feats_lengths"])
    m = dict(_prep_shared_inputs(inputs))
    m.update(_prep_core_inputs(inputs, core, row_map))
    sim = CoreSim(nc, require_finite=False, require_nnan=False)
    for k, v in m.items():
        sim.tensor(k)[:] = v
    sim.simulate()
    return {k: np.array(sim.tensor(k)) for k in ("out_stats", "out_pres")}


def kernel(**inputs) -> np.ndarray:
    out, _ = _run(inputs, trace=False)
    return out
